# revision 26
# baseline (speedup 1.0000x reference)
"""Trainium2 Bass kernel for nn_DecoderCell (LFADS decoder cell).

Strategy: pure data parallel over 8 NeuronCores (8192 batch rows each).
On-chip layout is fully transposed ([feature, batch]): batch rides the free
dim (512-wide tiles), gate features ride the partitions. All matmuls are
fp32r (full-rate at free-dim >= 256) with the small weights stationary
(host pre-transposed) and activations streaming. Biases are folded into the
matmuls via ones-row augmentation of the K dim. Sigmoid is synthesized from
tanh (one ACT table set: Exp+Tanh) with the affine absorbed into fused
scalar_tensor_tensor ops.

Host side only transposes/shards numpy arrays; all compute is on device.
"""

import numpy as np

import concourse.bass as bass
import concourse.tile as tile
from concourse import bacc, mybir
from concourse.bass_utils import run_bass_kernel_spmd

# ---- problem constants (hardcoded; kernel.py must be self-contained) ----
B = 65536
N_CORES = 8
ROWS = B // N_CORES          # 8192 rows per core
NB = 256                     # batch tile (free dim)
NT = ROWS // NB              # 16 tiles per core

GEN = 200
CON = 128
CO = 4
LAT = 64
CIE = 128                    # CI_ENC_DIM
EXT = 16
CLIP = 5.0
GEN_IN = EXT + CO            # 20
CON_IN = 2 * CIE + LAT       # 320
STATE = 420

F32 = mybir.dt.float32
F32R = mybir.dt.float32r
BF16 = mybir.dt.bfloat16
AF = mybir.ActivationFunctionType
ALU = mybir.AluOpType


# packed-weight column layout: name -> (rows, cols, col_offset)
_WCOLS = {}
_off = 0
for _nm, _p, _f in (
    ("cwA", 128, 384), ("cwB", 128, 384), ("cwC", 65, 384), ("cwH", 128, 384),
    ("cbHN", 1, 128), ("gwI", 21, 600), ("gwHA", 128, 600), ("gwHB", 73, 600),
    ("coW", 128, 8), ("coB", 1, 8), ("facWA", 128, 64), ("facWB", 72, 64),
    ("ones", 1, 512),
):
    _WCOLS[_nm] = (_p, _f, _off)
    _off += _f
WPACK_COLS = _off


def _f(ap):

    """View an fp32r AP as plain fp32 for DVE/ACT/DMA use."""
    return ap.bitcast(F32)


def build_decoder(nc: bass.Bass, tc: tile.TileContext, ctx, ins: dict, outs: dict,
                  rows: int = ROWS, nb: int = NB):
    """Emit the per-core program. `ins`/`outs` map name -> DRAM AP.

    Super-tiles of 2*nb rows are loaded/stored with packed DMAs; compute
    runs on nb-wide subtiles. All DRAM layouts are host-packed tile-major.
    """
    NB = nb           # noqa: N806
    NB2 = 2 * nb      # noqa: N806 — super-tile width
    NST = rows // NB2  # noqa: N806

    wp = ctx.enter_context(tc.tile_pool(name="wp", bufs=1))
    lp = ctx.enter_context(tc.tile_pool(name="lp", bufs=4))
    gp = ctx.enter_context(tc.tile_pool(name="gp", bufs=5))
    op = ctx.enter_context(tc.tile_pool(name="op", bufs=4))
    pp = ctx.enter_context(tc.tile_pool(name="pp", bufs=8, space="PSUM"))

    # ---- persistent weights in SBUF: one packed tile, one DMA ----
    wsb = wp.tile([128, WPACK_COLS], F32R, name="wsb")
    nc.sync.dma_start(wsb[:], ins["wpack"][:])

    def wv(name):
        p, f, c0 = _WCOLS[name]
        return wsb[0:p, c0:c0 + f]

    cwA, cwB, cwC, cwH = wv("cwA"), wv("cwB"), wv("cwC"), wv("cwH")
    cbHN, gwI, gwHA, gwHB = wv("cbHN"), wv("gwI"), wv("gwHA"), wv("gwHB")
    coW, coB, facWA, facWB = wv("coW"), wv("coB"), wv("facWA"), wv("facWB")
    ones = wv("ones")

    mm = nc.tensor.matmul
    CH = 2  # super-tiles per pipeline chunk

    def stage_load(st):
        c2 = slice(st * NB2, (st + 1) * NB2)
        c4 = slice(st * 2 * NB2, (st + 1) * 2 * NB2)
        grp1 = lp.tile([128, 2 * NB2], F32R, name="grp1")   # [ci0 | ci1]
        nc.sync.dma_start(grp1[:], ins["grp1"][:, c4])
        grp2 = lp.tile([128, 2 * NB2], F32R, name="grp2")   # [con_s | gen0]
        nc.sync.dma_start(grp2[:], ins["grp2"][:, c4])
        grp3 = lp.tile([73, 2 * NB2], F32R, name="grp3")    # [gen1 | facp]
        nc.sync.dma_start(grp3[:], ins["grp3"][:, c4])
        gin = gp.tile([68, NB2], F32R, name="gin")
        nc.sync.dma_start(gin[4:68, :], ins["ginb3"][:, c2])
        epsv = gp.tile([CO, NB2], F32, name="epsv")
        nc.sync.dma_start(epsv[:], _f(ins["ginb3"][28:32, c2]))
        og1 = op.tile([128, 2 * NB2], F32R, name="og1")     # [genpA | conp]
        og2 = op.tile([72, NB2], F32R, name="og2")          # genpB
        fct = op.tile([64, NB2], F32, name="fct")           # factor
        subs = []
        for s in range(2):
            cs = slice(s * NB, (s + 1) * NB)

            def blk(base, s=s):
                return slice(base + s * NB, base + (s + 1) * NB)

            subs.append(dict(
                cs=cs,
                ci0=grp1[0:128, blk(0)], ci1=grp1[0:128, blk(NB2)],
                con_s=grp2[0:128, blk(0)], gen0=grp2[0:128, blk(NB2)],
                gen1=grp3[0:73, blk(0)], facp=grp3[0:65, blk(NB2)],
                gin=gin, g_in=gin[0:GEN_IN + 1, cs], epsv=epsv[0:CO, cs],
                genpA=og1[0:128, blk(0)], conp=og1[0:128, blk(NB2)],
                genpB=og2[0:72, cs], fct=fct,
            ))
        io = dict(st=st, c2=c2, c4=c4, og1=og1, og2=og2, fct=fct, gin=gin)
        return subs, io

    def stage_con(t):
        p_crz = pp.tile([128, 2 * NB], F32, name="p_crz", tag="pp")
        p_cn = pp.tile([128, 2 * NB], F32, name="p_cn", tag="pp")
        for half, c0 in ((slice(0, NB), 0), (slice(NB, 2 * NB), 128)):
            mm(p_crz[:, half], cwA[:, c0:c0 + 128], t["ci0"], start=True, stop=False)
            mm(p_crz[:, half], cwB[:, c0:c0 + 128], t["ci1"], start=False, stop=False)
            mm(p_crz[:, half], cwC[:, c0:c0 + 128], t["facp"], start=False, stop=False)
            mm(p_crz[:, half], cwH[:, c0:c0 + 128], t["con_s"], start=False, stop=True)
        mm(p_cn[:, 0:NB], cwA[:, 256:384], t["ci0"], start=True, stop=False)
        mm(p_cn[:, 0:NB], cwB[:, 256:384], t["ci1"], start=False, stop=False)
        mm(p_cn[:, 0:NB], cwC[:, 256:384], t["facp"], start=False, stop=True)
        mm(p_cn[:, NB:2 * NB], cwH[:, 256:384], t["con_s"], start=True, stop=False)
        mm(p_cn[:, NB:2 * NB], cbHN[:], ones[:, 0:NB], start=False, stop=True)

        t_crz = gp.tile([128, 2 * NB], BF16, name="t_crz")
        nc.scalar.activation(t_crz[:], p_crz[:], AF.Tanh, scale=0.5)
        tp_c = gp.tile([128, NB], F32, name="tp_c")
        nc.vector.scalar_tensor_tensor(  # (1+tanh_r)*h_n == 2*r*h_n
            tp_c[:], t_crz[:, 0:NB], 1.0, p_cn[:, NB:2 * NB],
            op0=ALU.add, op1=ALU.mult)
        u_c = gp.tile([128, NB], F32, name="u_c")
        nc.vector.scalar_tensor_tensor(  # 0.5*(2*r*h_n) + i_n
            u_c[:], tp_c[:], 0.5, p_cn[:, 0:NB], op0=ALU.mult, op1=ALU.add)
        n_c = gp.tile([128, NB], BF16, name="n_c")
        nc.scalar.activation(n_c[:], u_c[:], AF.Tanh)
        d_c = gp.tile([128, NB], BF16, name="d_c")
        nc.gpsimd.tensor_sub(d_c[:], _f(t["con_s"]), n_c[:])
        e_c = gp.tile([128, NB], BF16, name="e_c")
        nc.vector.scalar_tensor_tensor(  # (1+tanh_z)*(h-n)
            e_c[:], t_crz[:, NB:2 * NB], 1.0, d_c[:], op0=ALU.add, op1=ALU.mult)
        cpre = gp.tile([128, NB], BF16, name="cpre")
        nc.vector.scalar_tensor_tensor(  # n + 0.5*e
            cpre[:], e_c[:], 0.5, n_c[:], op0=ALU.mult, op1=ALU.add)
        nc.gpsimd.tensor_scalar(  # clip into packed output
            t["conp"], cpre[:], CLIP, -CLIP, op0=ALU.min, op1=ALU.max)

    def stage_co(t):
        cs = t["cs"]
        gin = t["gin"]
        p_co = pp.tile([CO, 2 * NB], F32, name="p_co", tag="pp")
        mm(p_co[:, 0:NB], coW[:, 0:CO], t["conp"], start=True, stop=False)
        mm(p_co[:, 0:NB], coB[:, 0:CO], ones[:, 0:NB], start=False, stop=True)
        mm(p_co[:, NB:2 * NB], coW[:, CO:2 * CO], t["conp"], start=True, stop=False)
        mm(p_co[:, NB:2 * NB], coB[:, CO:2 * CO], ones[:, 0:NB],
           start=False, stop=True)
        q_co = gp.tile([CO, NB], F32, name="q_co")
        stdt = gp.tile([CO, NB], F32, name="stdt")
        nc.scalar.activation(stdt[:], p_co[:, NB:2 * NB], AF.Exp, scale=0.5)
        nc.vector.tensor_mul(q_co[:], stdt[:], t["epsv"])           # std*eps
        nc.scalar.copy(gin[64:68, cs], p_co[:, 0:NB])               # co_mean
        # co_std into the (now unused) eps slot for the output DMA
        nc.vector.tensor_copy(gin[32:36, cs], stdt[:])
        # con_out = mean + std*eps (mean read from PSUM: offset-exempt)
        nc.vector.tensor_add(gin[0:CO, cs], q_co[:], p_co[:, 0:NB])

    def stage_gen(t):
        for (msz, m0, h_tile, outp) in (
            (128, 0, t["gen0"], t["genpA"]),
            (72, 128, t["gen1"][0:72, :], t["genpB"]),
        ):
            p_grz = pp.tile([msz, 2 * NB], F32, name=f"p_grz{m0}", tag="pp")
            p_gn = pp.tile([msz, 2 * NB], F32, name=f"p_gn{m0}", tag="pp")
            for half, c0 in ((slice(0, NB), m0), (slice(NB, 2 * NB), 200 + m0)):
                mm(p_grz[:, half], gwI[:, c0:c0 + msz], t["g_in"],
                   start=True, stop=False)
                mm(p_grz[:, half], gwHA[:, c0:c0 + msz], t["gen0"],
                   start=False, stop=False)
                mm(p_grz[:, half], gwHB[:, c0:c0 + msz], t["gen1"],
                   start=False, stop=True)
            mm(p_gn[:, 0:NB], gwI[:, 400 + m0:400 + m0 + msz], t["g_in"],
               start=True, stop=True)
            mm(p_gn[:, NB:2 * NB], gwHA[:, 400 + m0:400 + m0 + msz], t["gen0"],
               start=True, stop=False)
            mm(p_gn[:, NB:2 * NB], gwHB[:, 400 + m0:400 + m0 + msz], t["gen1"],
               start=False, stop=True)

            t_grz = gp.tile([msz, 2 * NB], BF16, name=f"t_grz{m0}", tag="t_grz")
            nc.scalar.activation(t_grz[:], p_grz[:], AF.Tanh, scale=0.5)
            tp_g = gp.tile([msz, NB], F32, name=f"tp_g{m0}", tag="tp_g")
            nc.vector.scalar_tensor_tensor(
                tp_g[:], t_grz[:, 0:NB], 1.0, p_gn[:, NB:2 * NB],
                op0=ALU.add, op1=ALU.mult)
            u_g = gp.tile([msz, NB], F32, name=f"u_g{m0}", tag="u_g")
            nc.vector.scalar_tensor_tensor(
                u_g[:], tp_g[:], 0.5, p_gn[:, 0:NB], op0=ALU.mult, op1=ALU.add)
            n_g = gp.tile([msz, NB], BF16, name=f"n_g{m0}", tag="n_g")
            nc.scalar.activation(n_g[:], u_g[:], AF.Tanh)
            d_g = gp.tile([msz, NB], BF16, name=f"d_g{m0}", tag="d_g")
            nc.gpsimd.tensor_sub(d_g[:], _f(h_tile), n_g[:])
            e_g = gp.tile([msz, NB], BF16, name=f"e_g{m0}", tag="e_g")
            nc.vector.scalar_tensor_tensor(
                e_g[:], t_grz[:, NB:2 * NB], 1.0, d_g[:],
                op0=ALU.add, op1=ALU.mult)
            gpre = gp.tile([msz, NB], BF16, name=f"gpre{m0}", tag="gpre")
            nc.vector.scalar_tensor_tensor(
                gpre[:], e_g[:], 0.5, n_g[:], op0=ALU.mult, op1=ALU.add)
            nc.gpsimd.tensor_scalar(
                outp, gpre[:], CLIP, -CLIP, op0=ALU.min, op1=ALU.max)

    def stage_fac(t):
        p_f = pp.tile([LAT, NB], F32, name="p_f", tag="pp")
        mm(p_f[:], facWA[:], t["genpA"], start=True, stop=False)
        mm(p_f[:], facWB[:], t["genpB"], start=False, stop=True)
        nc.scalar.copy(t["fct"][:, t["cs"]], p_f[:])

    def stage_store(io):
        nc.sync.dma_start(outs["og1"][:, io["c4"]], _f(io["og1"][:]))
        nc.sync.dma_start(outs["og2"][:, io["c2"]], _f(io["og2"][:]))
        nc.sync.dma_start(outs["fct"][:, io["c2"]], io["fct"][:])
        nc.sync.dma_start(outs["ginout"][:, io["c2"]], _f(io["gin"][0:68, :]))

    assert NST % CH == 0
    for ch in range(NST // CH):
        subs, ios = [], []
        for i in range(CH):
            s2, io = stage_load(ch * CH + i)
            subs.extend(s2)
            ios.append(io)
        for t in subs:
            stage_con(t)
        for t in subs:
            stage_co(t)
        for t in subs:
            stage_gen(t)
        for t in subs:
            stage_fac(t)
        for io in ios:
            stage_store(io)


def _weight_arrays(gen_w_ih, gen_w_hh, gen_b_ih, gen_b_hh,
                   con_w_ih, con_w_hh, con_b_ih, con_b_hh, co_w, co_b, fac_w):
    """Host-side weight prep: transpose + bias-row augmentation."""
    f = np.float32
    cw = np.ascontiguousarray(con_w_ih.T, dtype=f)      # [320, 384]
    cbias = con_b_ih.astype(f).copy()
    cbias[:256] += con_b_hh[:256].astype(f)             # rz combined; n = b_ih only
    cwC = np.concatenate([cw[256:320], cbias[None, :]], axis=0)
    gw = np.ascontiguousarray(gen_w_ih.T, dtype=f)      # [20, 600]
    gbias = gen_b_ih.astype(f).copy()
    gbias[:400] += gen_b_hh[:400].astype(f)
    gwI = np.concatenate([gw, gbias[None, :]], axis=0)  # [21, 600]
    gh = np.ascontiguousarray(gen_w_hh.T, dtype=f)      # [200, 600]
    ghb = np.zeros((1, 600), dtype=f)
    ghb[0, 400:] = gen_b_hh[400:]
    gwHB = np.concatenate([gh[128:200], ghb], axis=0)   # [73, 600]
    nrm = np.maximum(np.linalg.norm(fac_w.astype(np.float64), axis=1,
                                    keepdims=True), 1e-12)
    facn = np.ascontiguousarray((fac_w / nrm).T, dtype=f)  # [200, 64]
    parts = {
        "cwA": cw[0:128], "cwB": cw[128:256], "cwC": cwC,
        "cwH": np.ascontiguousarray(con_w_hh.T, dtype=f),
        "cbHN": con_b_hh[256:384].astype(f).reshape(1, 128),
        "gwI": gwI, "gwHA": gh[0:128], "gwHB": gwHB,
        "coW": np.ascontiguousarray(co_w.T, dtype=f),
        "coB": co_b.astype(f).reshape(1, 8),
        "facWA": facn[0:128], "facWB": facn[128:200],
        "ones": np.ones((1, 512), dtype=f),
    }
    wpack = np.zeros((128, WPACK_COLS), dtype=f)
    for nm, (p, fc, c0) in _WCOLS.items():
        wpack[0:p, c0:c0 + fc] = parts[nm]
    return {"wpack": wpack}


_CACHED = {}


def _build_nc(rows=ROWS, nb=NB):
    if (rows, nb) in _CACHED:
        return _CACHED[(rows, nb)]
    from contextlib import ExitStack

    nc = bacc.Bacc("TRN2", target_bir_lowering=False, debug=False,
                   num_devices=N_CORES)
    names_in = {
        "grp1": [128, 2 * rows], "grp2": [128, 2 * rows],
        "grp3": [73, 2 * rows], "ginb3": [64, rows],
        "wpack": [128, WPACK_COLS],
    }
    ins = {k: nc.dram_tensor(k, v, F32R, kind="ExternalInput").ap()
           for k, v in names_in.items()}
    outs = {
        "og1": nc.dram_tensor("og1", [128, 2 * rows], F32,
                              kind="ExternalOutput").ap(),
        "og2": nc.dram_tensor("og2", [72, rows], F32,
                              kind="ExternalOutput").ap(),
        "fct": nc.dram_tensor("fct", [64, rows], F32,
                              kind="ExternalOutput").ap(),
        "ginout": nc.dram_tensor("ginout", [68, rows], F32,
                                 kind="ExternalOutput").ap(),
    }
    with tile.TileContext(nc) as tc:
        with ExitStack() as ctx:
            build_decoder(nc, tc, ctx, ins, outs, rows=rows, nb=nb)
    nc.compile()
    _CACHED[(rows, nb)] = nc
    return nc


def pack_inputs(x, h0, eps, rows, nb=NB):
    """Host-side tile-major packing of one core's activations."""
    f = np.float32
    nb2 = 2 * nb
    nst = rows // nb2
    one = np.ones((1, rows), dtype=f)

    def inter(a, b):
        # [p, rows] x2 -> [p, 2*rows] with per-super-tile [a_block | b_block]
        p = a.shape[0]
        out = np.empty((p, 2 * rows), dtype=f)
        av = a.reshape(p, nst, nb2)
        bv = b.reshape(p, nst, nb2)
        ov = out.reshape(p, nst, 2, nb2)
        ov[:, :, 0, :] = av
        ov[:, :, 1, :] = bv
        return out

    xT = x.T  # [272, rows]
    grp1 = inter(np.ascontiguousarray(xT[0:128]), np.ascontiguousarray(xT[128:256]))
    grp2 = inter(np.ascontiguousarray(h0[:, 200:328].T),
                 np.ascontiguousarray(h0[:, 0:128].T))
    gen1 = np.concatenate([h0[:, 128:200].T, one], axis=0)          # [73, rows]
    facp = np.concatenate([h0[:, 356:420].T, one,
                           np.zeros((8, rows), dtype=f)], axis=0)   # [73, rows]
    grp3 = inter(np.ascontiguousarray(gen1), facp)
    ginb3 = np.concatenate([
        x[:, 256:272].T, one, np.zeros((11, rows), dtype=f), eps.T,
        np.zeros((32, rows), dtype=f),
    ], axis=0)                                                       # [64, rows]
    return {"grp1": grp1, "grp2": grp2, "grp3": grp3,
            "ginb3": np.ascontiguousarray(ginb3)}


def unpack_outputs(res, rows, nb=NB):
    """Invert the packed og1/og2/ginout layouts into [rows, 420]."""
    nb2 = 2 * nb
    nst = rows // nb2
    out = np.empty((rows, STATE), dtype=np.float32)
    og1 = res["og1"].reshape(128, nst, 2, nb2)   # [genpA | conp]
    genpA = og1[:, :, 0, :].reshape(128, rows)
    conp = og1[:, :, 1, :].reshape(128, rows)
    gin = res["ginout"]                          # [68, rows]
    out[:, 0:128] = genpA.T
    out[:, 128:200] = res["og2"].T
    out[:, 200:328] = conp.T
    out[:, 328:332] = gin[64:68].T
    out[:, 332:336] = gin[32:36].T
    out[:, 336:356] = gin[0:20].T
    out[:, 356:420] = res["fct"].T
    return out


def kernel(x, h0, eps, gen_w_ih, gen_w_hh, gen_b_ih, gen_b_hh,
           con_w_ih, con_w_hh, con_b_ih, con_b_hh, co_w, co_b, fac_w,
           **run_kwargs):
    x = np.asarray(x, dtype=np.float32)
    h0 = np.asarray(h0, dtype=np.float32)
    eps = np.asarray(eps, dtype=np.float32)
    w = _weight_arrays(gen_w_ih, gen_w_hh, gen_b_ih, gen_b_hh,
                       con_w_ih, con_w_hh, con_b_ih, con_b_hh,
                       co_w, co_b, fac_w)
    nc = _build_nc()

    in_maps = []
    for c in range(N_CORES):
        r0, r1 = c * ROWS, (c + 1) * ROWS
        m = dict(w)
        m.update(pack_inputs(x[r0:r1], h0[r0:r1], eps[r0:r1], ROWS))
        in_maps.append(m)

    res = run_bass_kernel_spmd(nc, in_maps, core_ids=list(range(N_CORES)),
                               **run_kwargs)
    out = np.empty((B, STATE), dtype=np.float32)
    for c in range(N_CORES):
        out[c * ROWS:(c + 1) * ROWS] = unpack_outputs(res.results[c], ROWS)
    if run_kwargs:
        return out, res
    return out
